# revision 27
# baseline (speedup 1.0000x reference)
"""Trainium2 Bass kernel for nn_GCN (batched GCN + LSTM + actor/critic heads).

Strategy (8 NeuronCores, SPMD):
  - Stage 1 (data-parallel over B = bs*T = 512; 64 rows per core):
    algebraically-collapsed GCN layer 1 (rank-structured broadcast + relu),
    per-b node matmuls for layers 2/3, feature heads, fused FC layer, and the
    LSTM input-gate precompute Gx = out @ Wih.T (+ bias via a ones-row).
  - Gx shards are AllGathered (bf16) across the 8 cores, then transposed into
    gate-major layout with xbar DMA transposes.
  - Stage 2 (replicated on every core): the sequential LSTM over T=32 steps in
    transposed (gate-on-partition) form with bf16 Whh stationary weights; Gx is
    injected into PSUM with an identity-matmul accumulate. Gates are laid out
    (i, f, o, g) so one sigmoid covers i/f/o. Hidden states are stored bf16 and
    feed the next step and the actor/critic heads directly.
Host side only reshapes/transposes/casts weights and derives B-independent
small matrices (word embedding, normalized adjacency variants).
"""

import numpy as np
import ml_dtypes

import concourse.bacc as bacc
import concourse.bass as bass
import concourse.tile as tile
import concourse.mybir as mybir
from concourse.bass_utils import run_bass_kernel_spmd

FP32 = mybir.dt.float32
BF16 = mybir.dt.bfloat16
AF = mybir.ActivationFunctionType
ALU = mybir.AluOpType

NCORES = 8
BS, T, NOBJ = 16, 32, 100
GLOVE, GEMB, ADIM, GCN_DIM, H, ASPACE, RES = 300, 64, 10, 512, 512, 6, 512
B = BS * T          # 512
BLOC = B // NCORES  # 64 rows per core
BBLK = 8            # b-block size for the Z/GCN pipeline
C1 = 256            # GCN hidden channels
KCH = H // 128      # 4
GCH = 4 * H // 128  # 16

_CACHE = {}
GX_VIA_MM = True


def _build_program():
    nc = bacc.Bacc("TRN2", target_bir_lowering=False, debug=False,
                   num_devices=NCORES)

    def din(name, shape, dt=FP32):
        return nc.dram_tensor(name, shape, dt, kind="ExternalInput").ap()

    def dout(name, shape, dt=FP32):
        return nc.dram_tensor(name, shape, dt, kind="ExternalOutput").ap()

    # --- per-core (sharded) batch inputs (pre-laid [128, chunks*cols]) ---
    featsT_d = din("featsT", [128, KCH * BLOC])
    tgtT_d = din("tgtT", [128, 3 * BLOC])
    actT_d = din("actT", [ASPACE, BLOC])
    # --- replicated weights / derived (gate dims permuted to i,f,o,g) ---
    w0fT_d = din("w0fT", [128, KCH * C1])
    wgT_d = din("wgT", [128, 3 * GEMB])
    waT_d = din("waT", [ASPACE, ADIM])
    vpT_d = din("vpT", [128, 2 * NOBJ], BF16)
    w1T_d = din("w1T", [128, 2 * C1], BF16)
    apT_d = din("apT", [NOBJ, NOBJ], BF16)
    a_d = din("a_mat", [NOBJ, NOBJ])
    w2r_d = din("w2rep", [NOBJ, C1])
    wfmT_d = din("wfmT", [NOBJ, GCN_DIM])
    wfT_d = din("wfT", [128, 10 * H])
    wihT_d = din("wihT", [H, 4 * H])
    bihrow_d = din("bihrow", [1, 4 * H])           # bih + bhh (permuted)
    whhT_d = din("whhT", [128, KCH * 4 * H], BF16)
    eye_d = din("eye128", [128, 128], BF16)
    wa1T_d = din("wa1T", [128, KCH * 128], BF16)
    wa2T_d = din("wa2T", [128, ASPACE])
    wc1T_d = din("wc1T", [128, KCH * 128], BF16)
    wc2T_d = din("wc2T", [128, 1])
    bg_d = din("bg", [GEMB, 1])
    ba_d = din("ba", [ADIM, 1])
    bfm_d = din("bfm", [128, KCH])
    bf_d = din("bf", [128, KCH])
    ba1_d = din("ba1", [128, 1])
    ba2_d = din("ba2", [ASPACE, 1])
    bc1_d = din("bc1", [128, 1])
    bc2_d = din("bc2", [1, 1])

    actorT_o = dout("actorT", [ASPACE, B])
    criticT_o = dout("criticT", [1, B])
    hT_o = dout("hT", [128, KCH * BS])
    cT_o = dout("cT", [128, KCH * BS])

    with tile.TileContext(nc) as tc:
        with (
            tc.tile_pool(name="persist", bufs=1) as pp,
            tc.tile_pool(name="wstream", bufs=2) as wsp,
            tc.tile_pool(name="zblk", bufs=3) as zbp,
            tc.tile_pool(name="small", bufs=2) as smp,
            tc.tile_pool(name="psum", bufs=2, space="PSUM") as ps,
            tc.tile_pool(name="dram", bufs=1, space="DRAM") as drp,
        ):
            def load(dram_ap, name, shape, dt=FP32):
                t_ = pp.tile(shape, dt, name=name, tag=name)
                dst = t_[:]
                if len(shape) == 3:
                    dst = dst.rearrange("p a b -> p (a b)")
                nc.sync.dma_start(dst, dram_ap)
                return t_

            def ps_sm(p_, f_):
                return ps.tile([p_, f_], FP32, name="ps_sm", tag="ps_big",
                               bufs=8)

            def ps_big(p_, f_, f2=None):
                shp = [p_, f_] if f2 is None else [p_, f_, f2]
                return ps.tile(shp, FP32, name="ps_big", tag="ps_big", bufs=8)

            # stage-1-critical loads first
            featsT = load(featsT_d, "featsT_s", [128, KCH, BLOC])
            w0fT = load(w0fT_d, "w0fT_s", [128, KCH, C1])
            vpT = load(vpT_d, "vpT_s", [128, 2, NOBJ], dt=BF16)
            w1T = load(w1T_d, "w1T_s", [128, 2, C1], dt=BF16)
            apT = load(apT_d, "apT_s", [NOBJ, NOBJ], dt=BF16)
            w2rep = load(w2r_d, "w2rep_s", [NOBJ, C1])
            tgtT = load(tgtT_d, "tgtT_s", [128, 3, BLOC])
            wgT = load(wgT_d, "wgT_s", [128, 3, GEMB])
            actT = load(actT_d, "actT_s", [ASPACE, BLOC])
            waT = load(waT_d, "waT_s", [ASPACE, ADIM])
            a_sb = load(a_d, "a_s", [NOBJ, NOBJ])
            wfmT = load(wfmT_d, "wfmT_s", [NOBJ, GCN_DIM])
            bg_s = load(bg_d, "bg_s", [GEMB, 1])
            ba_s = load(ba_d, "ba_s", [ADIM, 1])
            bfm_s = load(bfm_d, "bfm_s", [128, KCH])
            bf_s = load(bf_d, "bf_s", [128, KCH])

            # ---------- S1: u = feats @ W0f.T -> uT (fp32) -> ubf (bf16) ----
            uT = pp.tile([128, 2, BLOC], FP32, name="uT", tag="uT")
            for m in range(2):
                ups = ps_sm(128, BLOC)
                for k in range(KCH):
                    nc.tensor.matmul(ups[:], w0fT[:, k, bass.ts(m, 128)],
                                     featsT[:, k, :],
                                     start=(k == 0), stop=(k == KCH - 1))
                nc.scalar.copy(uT[:, m, :], ups[:])
            ubf = pp.tile([128, 2, BLOC], BF16, name="ubf", tag="ubf")
            nc.vector.tensor_copy(ubf[:], uT[:])

            # ---------- S2/S3: glove_e, act_e ----------
            gloveT = pp.tile([GEMB, BLOC], FP32, name="gloveT", tag="gloveT")
            gps = ps_sm(GEMB, BLOC)
            for k in range(3):
                nc.tensor.matmul(gps[:], wgT[:, k, :], tgtT[:, k, :],
                                 start=(k == 0), stop=(k == 2))
            nc.scalar.activation(gloveT[:], gps[:], AF.Relu, bias=bg_s[:])

            actE = pp.tile([ADIM, BLOC], FP32, name="actE", tag="actE")
            aps2 = ps_sm(ADIM, BLOC)
            nc.tensor.matmul(aps2[:], waT[:], actT[:], start=True, stop=True)
            nc.scalar.activation(actE[:], aps2[:], AF.Relu, bias=ba_s[:])

            # ---------- late loads (LSTM / heads weights) ----------
            wfT = load(wfT_d, "wfT_s", [128, 10, H])
            whhT = load(whhT_d, "whhT_s", [128, KCH, 4 * H], dt=BF16)
            eye = load(eye_d, "eye_s", [128, 128], dt=BF16)
            bihrow = load(bihrow_d, "bihrow_s", [1, 4 * H])
            wa1T = load(wa1T_d, "wa1T_s", [128, KCH, 128], dt=BF16)
            wa2T = load(wa2T_d, "wa2T_s", [128, ASPACE])
            wc1T = load(wc1T_d, "wc1T_s", [128, KCH, 128], dt=BF16)
            wc2T = load(wc2T_d, "wc2T_s", [128, 1])
            ba1_s = load(ba1_d, "ba1_s", [128, 1])
            ba2_s = load(ba2_d, "ba2_s", [ASPACE, 1])
            bc1_s = load(bc1_d, "bc1_s", [128, 1])
            bc2_s = load(bc2_d, "bc2_s", [1, 1])
            ones_sb = pp.tile([1, BLOC], FP32, name="ones_sb", tag="ones_sb")
            nc.vector.memset(ones_sb[:], 1.0)
            wih_sb = []
            for k in range(KCH):
                wih_sb.append(load(wihT_d[bass.ts(k, 128), :], f"wih{k}",
                                   [128, 4 * H]))

            # ---------- S4..S9 pipelined over 4 t-chunks of 8 steps --------
            # Local b for chunk ck: {s*32 + 8*ck + i : s in 0..1, i in 0..7}.
            # Each chunk runs GCN layers 2/3, heads-fusion, Gx, and its own
            # AllGather + gate-transposes so the LSTM can start after chunk 0.
            # Local b columns are chunk-major: col = ck*16 + s*8 + t8
            # (host permutes featsT/tgtT/actT accordingly), so every chunk's
            # 16 columns are contiguous.
            NCK, TCK = 4, T // 4
            CW = 2 * TCK                       # 16 columns per chunk
            mcol = pp.tile([NOBJ, BLOC], FP32, name="mcol", tag="mcol")
            x3T = pp.tile([NOBJ, BLOC], FP32, name="x3T", tag="x3T")
            gcneT = pp.tile([128, KCH, BLOC], FP32, name="gcneT", tag="gcneT")
            outT = pp.tile([128, KCH, BLOC], FP32, name="outT", tag="outT")
            gxTc = []
            for ck in range(NCK):
                csl = bass.ts(ck, CW)
                zt = zbp.tile([128, 2, CW, NOBJ], BF16, name="zt", tag="zt")
                for cc in range(2):
                    zpre = smp.tile([128, CW, NOBJ], BF16, name="zpre",
                                    tag="zpre")
                    nc.gpsimd.tensor_tensor(
                        zpre[:],
                        ubf[:, cc, csl, None].to_broadcast([128, CW, NOBJ]),
                        vpT[:, cc, None, :].to_broadcast([128, CW, NOBJ]),
                        ALU.add)
                    nc.vector.tensor_scalar_max(zt[:, cc, :, :], zpre[:], 0.0)
                for bb in range(CW):
                    b = ck * CW + bb
                    pps = ps_big(NOBJ, C1)
                    nc.tensor.matmul(pps[:], zt[:, 0, bb, :],
                                     w1T[:, 0, :], start=True, stop=False)
                    nc.tensor.matmul(pps[:], zt[:, 1, bb, :],
                                     w1T[:, 1, :], start=False, stop=True)
                    psb = smp.tile([NOBJ, C1], BF16, name="psb", tag="psb")
                    nc.scalar.copy(psb[:], pps[:])
                    x2ps = ps_big(NOBJ, C1)
                    nc.tensor.matmul(x2ps[:], apT[:], psb[:],
                                     start=True, stop=True)
                    scr = smp.tile([NOBJ, C1], FP32, name="scr", tag="scr")
                    nc.vector.scalar_tensor_tensor(
                        scr[:], x2ps[:], 0.0, w2rep[:],
                        op0=ALU.max, op1=ALU.mult,
                        accum_out=mcol[:, b:b + 1])
                # S6: x3 chunk = relu(A @ mcol[:, chunk cols])
                x3ps = ps_sm(NOBJ, CW)
                nc.tensor.matmul(x3ps[:], a_sb[:], mcol[:, csl],
                                 start=True, stop=True)
                nc.scalar.activation(x3T[:, csl], x3ps[:], AF.Relu)
                # S7: gcn_e chunk
                for m in range(KCH):
                    gps2 = ps_sm(128, CW)
                    nc.tensor.matmul(gps2[:], wfmT[:, bass.ts(m, 128)],
                                     x3T[:, csl], start=True, stop=True)
                    nc.scalar.activation(gcneT[:, m, csl], gps2[:],
                                         AF.Identity, bias=bfm_s[:, m:m + 1])
                # S8: out chunk = relu(fused @ Wf.T + bf)
                chunks = (
                    [(featsT[:, k, csl], 128) for k in range(KCH)]
                    + [(gloveT[:, csl], GEMB), (actE[:, csl], ADIM)]
                    + [(gcneT[:, k, csl], 128) for k in range(KCH)]
                )
                for m in range(KCH):
                    ops = ps_sm(128, CW)
                    for i, (rhs, ksz) in enumerate(chunks):
                        nc.tensor.matmul(ops[:],
                                         wfT[0:ksz, i, bass.ts(m, 128)], rhs,
                                         start=(i == 0), stop=(i == 9))
                    nc.scalar.activation(outT[:, m, csl], ops[:], AF.Relu,
                                         bias=bf_s[:, m:m + 1])
                # S9: Gx chunk = out @ Wih.T + (bih+bhh)
                gxbf = smp.tile([CW, KCH, H], BF16, name="gxbf", tag="gxbf")
                gxps = [ps_big(CW, H) for _ in range(KCH)]
                for k in range(KCH):
                    for j in range(KCH):
                        nc.tensor.matmul(
                            gxps[j][:], outT[:, k, csl],
                            wih_sb[k][:, bass.ts(j, H)],
                            start=(k == 0), stop=False)
                for j in range(KCH):
                    nc.tensor.matmul(gxps[j][:], ones_sb[0:1, 0:CW],
                                     bihrow[0:1, bass.ts(j, H)],
                                     start=False, stop=True)
                    nc.vector.tensor_copy(gxbf[:, j, :], gxps[j][:])
                # AllGather this chunk + transpose into gate-major layout
                gx_in = drp.tile([CW, 4 * H], BF16, name=f"gx_in{ck}",
                                 tag=f"gx_in{ck}")
                gx_all = drp.tile([CW * NCORES, 4 * H], BF16,
                                  name=f"gx_all{ck}", tag=f"gx_all{ck}",
                                  addr_space="Shared")
                nc.sync.dma_start(gx_in[:],
                                  gxbf[:].rearrange("p a b -> p (a b)"))
                nc.gpsimd.collective_compute(
                    "AllGather", ALU.bypass,
                    replica_groups=[list(range(NCORES))],
                    ins=[gx_in.opt()], outs=[gx_all.opt()])
                gxt = pp.tile([128, GCH, BS, TCK], BF16, name=f"gxT{ck}",
                              tag=f"gxT{ck}")
                for gc in range(GCH):
                    eng = nc.sync if gc % 2 == 0 else nc.scalar
                    eng.dma_start_transpose(gxt[:, gc, :, :],
                                            gx_all[:, bass.ts(gc, 128)])
                gxTc.append(gxt)

            # ---------- S10: LSTM over T steps (transposed form) ----------
            # gate blocks (permuted): i = gc 0:4, f = 4:8, o = 8:12, g = 12:16
            c_sb = pp.tile([128, KCH, BS], FP32, name="c_sb", tag="c_sb")
            hs = pp.tile([128, T, KCH, BS], BF16, name="hs", tag="hs")
            hfin = pp.tile([128, KCH, BS], FP32, name="hfin", tag="hfin")
            nc.vector.memset(c_sb[:], 0.0)
            for t in range(T):
                gxslice = gxTc[t // TCK][:, :, :, t % TCK]
                gp = ps.tile([128, GCH, BS], FP32, name="gp", tag="ps_big",
                             bufs=8)
                # Gx_t (+ bias) seeds the accumulator via an identity matmul
                # (start=True), then the Whh matmuls accumulate. The o-gate
                # block (gc 8:12) is computed last so its matmuls overlap the
                # i/f/g nonlinearities and the c update.
                nc.tensor.matmul(gp[:], eye[:], gxslice,
                                 start=True, stop=(t == 0),
                                 skip_group_check=True)
                gc_order = [0, 1, 2, 3, 4, 5, 6, 7, 12, 13, 14, 15,
                            8, 9, 10, 11]
                for gc in (gc_order if t > 0 else []):
                    for k in range(KCH):
                        nc.tensor.matmul(
                            gp[:, gc, :],
                            whhT[:, k, bass.ts(gc, 128)],
                            hs[:, t - 1, k, :],
                            start=False, stop=(k == KCH - 1),
                            skip_group_check=True)
                gates = smp.tile([128, GCH, BS], FP32, name="gates",
                                 tag="gates")
                nc.scalar.activation(gates[:, 0:8, :], gp[:, 0:8, :],
                                     AF.Sigmoid)
                nc.scalar.activation(gates[:, 12:16, :], gp[:, 12:16, :],
                                     AF.Tanh)
                t1 = smp.tile([128, KCH, BS], FP32, name="t1", tag="t1")
                t2 = smp.tile([128, KCH, BS], FP32, name="t2", tag="t2")
                nc.vector.tensor_tensor(t1[:], gates[:, 4:8, :], c_sb[:],
                                        ALU.mult)
                nc.vector.tensor_tensor(t2[:], gates[:, 0:4, :],
                                        gates[:, 12:16, :], ALU.mult)
                nc.vector.tensor_tensor(c_sb[:], t1[:], t2[:], ALU.add)
                tanhc = smp.tile([128, KCH, BS], FP32, name="tanhc",
                                 tag="tanhc")
                nc.scalar.activation(tanhc[:], c_sb[:], AF.Tanh)
                nc.scalar.activation(gates[:, 8:12, :], gp[:, 8:12, :],
                                     AF.Sigmoid)
                nc.vector.tensor_tensor(hs[:, t, :, :], gates[:, 8:12, :],
                                        tanhc[:], ALU.mult)
                if t == T - 1:
                    nc.vector.tensor_tensor(hfin[:], gates[:, 8:12, :],
                                            tanhc[:], ALU.mult)

            # ---------- S11: actor / critic heads (replicated, all B) -------
            def head(w1t, b1, w2t, b2, odim, out_dram, nm):
                h1ps = ps_big(128, B)
                for k in range(KCH):
                    nc.tensor.matmul(
                        h1ps[:], w1t[:, k, :],
                        hs[:, :, k, :].rearrange("p t s -> p s t"),
                        start=(k == 0), stop=(k == KCH - 1))
                h1 = smp.tile([128, B], FP32, name=f"h1{nm}", tag="h1")
                nc.scalar.activation(h1[:], h1ps[:], AF.Relu, bias=b1[:])
                h2ps = ps_big(odim, B)
                nc.tensor.matmul(h2ps[:], w2t[:, 0:odim], h1[:],
                                 start=True, stop=True)
                h2 = smp.tile([odim, B], FP32, name=f"h2{nm}", tag="h2")
                nc.scalar.activation(h2[:], h2ps[:], AF.Relu, bias=b2[:])
                nc.sync.dma_start(out_dram[:], h2[:])

            head(wa1T, ba1_s, wa2T, ba2_s, ASPACE, actorT_o, "a")
            head(wc1T, bc1_s, wc2T, bc2_s, 1, criticT_o, "c")

            nc.sync.dma_start(hT_o[:], hfin[:].rearrange("p a b -> p (a b)"))
            nc.sync.dma_start(cT_o[:], c_sb[:].rearrange("p a b -> p (a b)"))

    nc.compile()
    return nc


def _gate_perm():
    """Permutation of the 4H gate dim from torch order (i,f,g,o) to the
    kernel's (i,f,o,g)."""
    idx = np.arange(4 * H).reshape(4, H)
    return np.concatenate([idx[0], idx[1], idx[3], idx[2]])


def _host_prep(inp):
    f32 = np.float32
    bf16 = ml_dtypes.bfloat16
    target = np.asarray(inp["target"], f32).reshape(B, GLOVE)
    feats = np.asarray(inp["input_"], f32).reshape(B, RES)
    aprob = np.asarray(inp["action_probs"], f32).reshape(B, ASPACE)
    A = np.asarray(inp["A"], np.float64)
    all_glove = np.asarray(inp["all_glove"], np.float64)

    Wword = np.asarray(inp["Wword"], np.float64)
    bword = np.asarray(inp["bword"], np.float64)
    W0 = np.asarray(inp["W0"], np.float64)
    word = all_glove @ Wword.T + bword
    r = A.sum(1)
    v = (A @ word) @ W0[:, RES:].T
    vpT = (v / r[:, None]).T.astype(f32)
    apT = (A * r[None, :]).T.astype(f32)

    perm = _gate_perm()

    def padrows(x, n):
        out = np.zeros((n, x.shape[1]), f32)
        out[:x.shape[0]] = x
        return out

    Wf = np.asarray(inp["Wf"], f32)
    wfT = np.ascontiguousarray(Wf.T)
    wfT_pack = np.zeros((1280, H), f32)
    wfT_pack[0:512] = wfT[0:512]
    wfT_pack[512:512 + GEMB] = wfT[512:512 + GEMB]
    wfT_pack[640:640 + ADIM] = wfT[576:586]
    wfT_pack[768:1280] = wfT[586:1098]

    col = lambda x: np.ascontiguousarray(np.asarray(x, f32).reshape(-1, 1))
    tr = lambda x: np.ascontiguousarray(np.asarray(x, f32).T)

    def lay(x, a):
        """[(a*128), b] -> [128, a*b] chunk-major partition layout."""
        b_ = x.shape[1]
        return np.ascontiguousarray(
            x.reshape(a, 128, b_).transpose(1, 0, 2).reshape(128, a * b_))

    bihrow = (np.asarray(inp["bih"], f32)
              + np.asarray(inp["bhh"], f32))[perm].reshape(1, -1)
    common = {
        "w0fT": lay(W0[:, :RES].T.astype(f32), KCH),
        "wgT": lay(padrows(tr(inp["Wg"]), 384), 3),
        "waT": tr(inp["Wa"]),
        "vpT": lay(np.ascontiguousarray(vpT), 2).astype(bf16),
        "w1T": lay(tr(inp["W1"]), 2).astype(bf16),
        "apT": np.ascontiguousarray(apT).astype(bf16),
        "a_mat": A.astype(f32),
        "w2rep": np.ascontiguousarray(
            np.broadcast_to(np.asarray(inp["W2"], f32)[0], (NOBJ, C1))),
        "wfmT": tr(inp["Wfm"]),
        "wfT": lay(wfT_pack, 10),
        "wihT": np.ascontiguousarray(tr(inp["Wih"])[:, perm]),
        "bihrow": np.ascontiguousarray(bihrow),
        "whhT": lay(np.ascontiguousarray(tr(inp["Whh"])[:, perm]),
                    KCH).astype(bf16),
        "eye128": np.eye(128, dtype=f32).astype(bf16),
        "wa1T": lay(tr(inp["Wa1"]), KCH).astype(bf16),
        "wa2T": tr(inp["Wa2"]),
        "wc1T": lay(tr(inp["Wc1"]), KCH).astype(bf16),
        "wc2T": tr(inp["Wc2"]),
        "bg": col(inp["bg"]), "ba": col(inp["ba"]),
        "bfm": np.ascontiguousarray(
            np.asarray(inp["bfm"], f32).reshape(KCH, 128).T),
        "bf": np.ascontiguousarray(
            np.asarray(inp["bf"], f32).reshape(KCH, 128).T),
        "ba1": col(inp["ba1"]), "ba2": col(inp["ba2"]),
        "bc1": col(inp["bc1"]), "bc2": col(inp["bc2"]),
    }
    # local column order is chunk-major: col = ck*16 + s*8 + t8
    permb = np.array([s * 32 + ck * 8 + t
                      for ck in range(4) for s in range(2) for t in range(8)])
    in_maps = []
    for c in range(NCORES):
        sl = slice(c * BLOC, (c + 1) * BLOC)
        m = dict(common)
        m["featsT"] = lay(np.ascontiguousarray(feats[sl].T[:, permb]), KCH)
        m["tgtT"] = lay(padrows(
            np.ascontiguousarray(target[sl].T[:, permb]), 384), 3)
        m["actT"] = np.ascontiguousarray(aprob[sl].T[:, permb])
        in_maps.append(m)
    return in_maps


def _run(inp, trace=False, **kw):
    if "nc" not in _CACHE:
        _CACHE["nc"] = _build_program()
    nc = _CACHE["nc"]
    in_maps = _host_prep(inp)
    res = run_bass_kernel_spmd(nc, in_maps, list(range(NCORES)), trace=trace,
                               **kw)
    r0 = res.results[0]
    actor = np.ascontiguousarray(r0["actorT"].T).reshape(BS, T, ASPACE)
    critic = np.ascontiguousarray(r0["criticT"].T).reshape(BS, T, 1)
    hT = r0["hT"].reshape(128, KCH, BS).transpose(2, 1, 0).reshape(1, BS, H)
    cT = r0["cT"].reshape(128, KCH, BS).transpose(2, 1, 0).reshape(1, BS, H)
    return (actor.astype(np.float32), critic.astype(np.float32),
            np.ascontiguousarray(hT).astype(np.float32),
            np.ascontiguousarray(cT).astype(np.float32)), res


def kernel(**inputs):
    outs, _ = _run(inputs, trace=False)
    return outs


# revision 35
# speedup vs baseline: 1.0462x; 1.0462x over previous
"""Trainium2 Bass kernel for nn_GCN (batched GCN + LSTM + actor/critic heads).

Strategy (8 NeuronCores, SPMD):
  - Stage 1 (data-parallel over B = bs*T = 512; 64 rows per core):
    algebraically-collapsed GCN layer 1 (rank-structured broadcast + relu),
    per-b node matmuls for layers 2/3, feature heads, fused FC layer, and the
    LSTM input-gate precompute Gx = out @ Wih.T (+ bias via a ones-row).
  - Gx shards are AllGathered (bf16) across the 8 cores, then transposed into
    gate-major layout with xbar DMA transposes.
  - Stage 2 (replicated on every core): the sequential LSTM over T=32 steps in
    transposed (gate-on-partition) form with bf16 Whh stationary weights; Gx is
    injected into PSUM with an identity-matmul accumulate. Gates are laid out
    (i, f, o, g) so one sigmoid covers i/f/o. Hidden states are stored bf16 and
    feed the next step and the actor/critic heads directly.
Host side only reshapes/transposes/casts weights and derives B-independent
small matrices (word embedding, normalized adjacency variants).
"""

import numpy as np
import ml_dtypes

import concourse.bacc as bacc
import concourse.bass as bass
import concourse.tile as tile
import concourse.mybir as mybir
from concourse.bass_utils import run_bass_kernel_spmd

FP32 = mybir.dt.float32
BF16 = mybir.dt.bfloat16
AF = mybir.ActivationFunctionType
ALU = mybir.AluOpType

NCORES = 8
BS, T, NOBJ = 16, 32, 100
GLOVE, GEMB, ADIM, GCN_DIM, H, ASPACE, RES = 300, 64, 10, 512, 512, 6, 512
B = BS * T          # 512
BLOC = B // NCORES  # 64 rows per core
BBLK = 8            # b-block size for the Z/GCN pipeline
C1 = 256            # GCN hidden channels
KCH = H // 128      # 4
GCH = 4 * H // 128  # 16

_CACHE = {}
GX_VIA_MM = True


def _build_program():
    nc = bacc.Bacc("TRN2", target_bir_lowering=False, debug=False,
                   num_devices=NCORES)

    def din(name, shape, dt=FP32):
        return nc.dram_tensor(name, shape, dt, kind="ExternalInput").ap()

    def dout(name, shape, dt=FP32):
        return nc.dram_tensor(name, shape, dt, kind="ExternalOutput").ap()

    # --- per-core (sharded) batch inputs (pre-laid [128, chunks*cols]) ---
    featsT_d = din("featsT", [128, KCH * BLOC])
    tgtT_d = din("tgtT", [128, 3 * BLOC])
    actT_d = din("actT", [ASPACE, BLOC])
    # --- replicated weights / derived (gate dims permuted to i,f,o,g) ---
    w0fT_d = din("w0fT", [128, KCH * C1])
    wgT_d = din("wgT", [128, 3 * GEMB])
    waT_d = din("waT", [ASPACE, ADIM])
    vpT_d = din("vpT", [128, 2 * NOBJ], BF16)
    w1T_d = din("w1T", [128, 2 * C1], BF16)
    apT_d = din("apT", [NOBJ, NOBJ], BF16)
    a_d = din("a_mat", [NOBJ, NOBJ])
    w2r_d = din("w2rep", [NOBJ, C1])
    wfmT_d = din("wfmT", [NOBJ, GCN_DIM])
    wfT_d = din("wfT", [128, 10 * H])
    wihT_d = din("wihT", [128, KCH * 4 * H], BF16)
    bfrep_d = din("bfrep", [2 * T // 4, H])        # bf replicated rows
    bih2c_d = din("bih2c", [128, GCH])             # bih+bhh (permuted) chunks
    whhT_d = din("whhT", [128, KCH * 4 * H], BF16)
    eye_d = din("eye128", [128, 128], BF16)
    wa1T_d = din("wa1T", [128, KCH * 128], BF16)
    wa2T_d = din("wa2T", [128, ASPACE])
    wc1T_d = din("wc1T", [128, KCH * 128], BF16)
    wc2T_d = din("wc2T", [128, 1])
    bg_d = din("bg", [GEMB, 1])
    ba_d = din("ba", [ADIM, 1])
    bfm_d = din("bfm", [128, KCH])
    ba1_d = din("ba1", [128, 1])
    ba2_d = din("ba2", [ASPACE, 1])
    bc1_d = din("bc1", [128, 1])
    bc2_d = din("bc2", [1, 1])

    actorT_o = dout("actorT", [ASPACE, B])
    criticT_o = dout("criticT", [1, B])
    hT_o = dout("hT", [128, KCH * BS])
    cT_o = dout("cT", [128, KCH * BS])

    with tile.TileContext(nc) as tc:
        with (
            tc.tile_pool(name="persist", bufs=1) as pp,
            tc.tile_pool(name="wstream", bufs=2) as wsp,
            tc.tile_pool(name="zblk", bufs=3) as zbp,
            tc.tile_pool(name="small", bufs=2) as smp,
            tc.tile_pool(name="psum", bufs=2, space="PSUM") as ps,
            tc.tile_pool(name="dram", bufs=1, space="DRAM") as drp,
        ):
            def load(dram_ap, name, shape, dt=FP32):
                t_ = pp.tile(shape, dt, name=name, tag=name)
                dst = t_[:]
                if len(shape) == 3:
                    dst = dst.rearrange("p a b -> p (a b)")
                nc.sync.dma_start(dst, dram_ap)
                return t_

            def ps_sm(p_, f_):
                return ps.tile([p_, f_], FP32, name="ps_sm", tag="ps_big",
                               bufs=8)

            def ps_big(p_, f_, f2=None):
                shp = [p_, f_] if f2 is None else [p_, f_, f2]
                return ps.tile(shp, FP32, name="ps_big", tag="ps_big", bufs=8)

            # stage-1-critical loads first
            featsT = load(featsT_d, "featsT_s", [128, KCH, BLOC])
            w0fT = load(w0fT_d, "w0fT_s", [128, KCH, C1])
            vpT = load(vpT_d, "vpT_s", [128, 2, NOBJ], dt=BF16)
            w1T = load(w1T_d, "w1T_s", [128, 2, C1], dt=BF16)
            apT = load(apT_d, "apT_s", [NOBJ, NOBJ], dt=BF16)
            w2rep = load(w2r_d, "w2rep_s", [NOBJ, C1])
            tgtT = load(tgtT_d, "tgtT_s", [128, 3, BLOC])
            wgT = load(wgT_d, "wgT_s", [128, 3, GEMB])
            actT = load(actT_d, "actT_s", [ASPACE, BLOC])
            waT = load(waT_d, "waT_s", [ASPACE, ADIM])
            a_sb = load(a_d, "a_s", [NOBJ, NOBJ])
            wfmT = load(wfmT_d, "wfmT_s", [NOBJ, GCN_DIM])
            bg_s = load(bg_d, "bg_s", [GEMB, 1])
            ba_s = load(ba_d, "ba_s", [ADIM, 1])
            bfm_s = load(bfm_d, "bfm_s", [128, KCH])

            # ---------- S1: u = feats @ W0f.T -> uT (fp32) -> ubf (bf16) ----
            uT = pp.tile([128, 2, BLOC], FP32, name="uT", tag="uT")
            for m in range(2):
                ups = ps_sm(128, BLOC)
                for k in range(KCH):
                    nc.tensor.matmul(ups[:], w0fT[:, k, bass.ts(m, 128)],
                                     featsT[:, k, :],
                                     start=(k == 0), stop=(k == KCH - 1))
                nc.scalar.copy(uT[:, m, :], ups[:])
            ubf = pp.tile([128, 2, BLOC], BF16, name="ubf", tag="ubf")
            nc.vector.tensor_copy(ubf[:], uT[:])

            # ---------- S2/S3: glove_e, act_e ----------
            gloveT = pp.tile([GEMB, BLOC], FP32, name="gloveT", tag="gloveT")
            gps = ps_sm(GEMB, BLOC)
            for k in range(3):
                nc.tensor.matmul(gps[:], wgT[:, k, :], tgtT[:, k, :],
                                 start=(k == 0), stop=(k == 2))
            nc.scalar.activation(gloveT[:], gps[:], AF.Relu, bias=bg_s[:])

            actE = pp.tile([ADIM, BLOC], FP32, name="actE", tag="actE")
            aps2 = ps_sm(ADIM, BLOC)
            nc.tensor.matmul(aps2[:], waT[:], actT[:], start=True, stop=True)
            nc.scalar.activation(actE[:], aps2[:], AF.Relu, bias=ba_s[:])

            # ---------- late loads (LSTM / heads weights) ----------
            wfT = load(wfT_d, "wfT_s", [128, 10, H])
            wihT = load(wihT_d, "wihT_s", [128, KCH, 4 * H], dt=BF16)
            whhT = load(whhT_d, "whhT_s", [128, KCH, 4 * H], dt=BF16)
            eye = load(eye_d, "eye_s", [128, 128], dt=BF16)
            bfrep = load(bfrep_d, "bfrep_s", [2 * T // 4, H])
            bih2c = load(bih2c_d, "bih2c_s", [128, GCH])
            wa1T = load(wa1T_d, "wa1T_s", [128, KCH, 128], dt=BF16)
            wa2T = load(wa2T_d, "wa2T_s", [128, ASPACE])
            wc1T = load(wc1T_d, "wc1T_s", [128, KCH, 128], dt=BF16)
            wc2T = load(wc2T_d, "wc2T_s", [128, 1])
            ba1_s = load(ba1_d, "ba1_s", [128, 1])
            ba2_s = load(ba2_d, "ba2_s", [ASPACE, 1])
            bc1_s = load(bc1_d, "bc1_s", [128, 1])
            bc2_s = load(bc2_d, "bc2_s", [1, 1])

            # ---------- S4..S9 pipelined over 4 t-chunks of 8 steps --------
            # Local b for chunk ck: {s*32 + 8*ck + i : s in 0..1, i in 0..7}.
            # Each chunk runs GCN layers 2/3, heads-fusion, Gx, and its own
            # AllGather + gate-transposes so the LSTM can start after chunk 0.
            # Local b columns are chunk-major: col = ck*16 + s*8 + t8
            # (host permutes featsT/tgtT/actT accordingly), so every chunk's
            # 16 columns are contiguous.
            NCK, TCK = 4, T // 4
            CW = 2 * TCK                       # 16 columns per chunk
            mcol = pp.tile([NOBJ, BLOC], FP32, name="mcol", tag="mcol")
            x3T = pp.tile([NOBJ, BLOC], FP32, name="x3T", tag="x3T")
            gcneT = pp.tile([128, KCH, BLOC], FP32, name="gcneT", tag="gcneT")
            gxTc = []
            for ck in range(NCK):
                csl = bass.ts(ck, CW)
                zt = zbp.tile([128, 2, CW, NOBJ], BF16, name="zt", tag="zt")
                for cc in range(2):
                    zpre = smp.tile([128, CW, NOBJ], BF16, name="zpre",
                                    tag="zpre")
                    nc.gpsimd.tensor_tensor(
                        zpre[:],
                        ubf[:, cc, csl, None].to_broadcast([128, CW, NOBJ]),
                        vpT[:, cc, None, :].to_broadcast([128, CW, NOBJ]),
                        ALU.add)
                    nc.vector.tensor_scalar_max(zt[:, cc, :, :], zpre[:], 0.0)
                for bb in range(CW):
                    b = ck * CW + bb
                    pps = ps_big(NOBJ, C1)
                    nc.tensor.matmul(pps[:], zt[:, 0, bb, :],
                                     w1T[:, 0, :], start=True, stop=False)
                    nc.tensor.matmul(pps[:], zt[:, 1, bb, :],
                                     w1T[:, 1, :], start=False, stop=True)
                    psb = smp.tile([NOBJ, C1], BF16, name="psb", tag="psb")
                    nc.scalar.copy(psb[:], pps[:])
                    x2ps = ps_big(NOBJ, C1)
                    nc.tensor.matmul(x2ps[:], apT[:], psb[:],
                                     start=True, stop=True)
                    scr = smp.tile([NOBJ, C1], FP32, name="scr", tag="scr")
                    nc.vector.scalar_tensor_tensor(
                        scr[:], x2ps[:], 0.0, w2rep[:],
                        op0=ALU.max, op1=ALU.mult,
                        accum_out=mcol[:, b:b + 1])
                # S6: x3 chunk = relu(A @ mcol[:, chunk cols])
                x3ps = ps_sm(NOBJ, CW)
                nc.tensor.matmul(x3ps[:], a_sb[:], mcol[:, csl],
                                 start=True, stop=True)
                nc.scalar.activation(x3T[:, csl], x3ps[:], AF.Relu)
                # S7: gcn_e chunk
                for m in range(KCH):
                    gps2 = ps_sm(128, CW)
                    nc.tensor.matmul(gps2[:], wfmT[:, bass.ts(m, 128)],
                                     x3T[:, csl], start=True, stop=True)
                    nc.scalar.activation(gcneT[:, m, csl], gps2[:],
                                         AF.Identity, bias=bfm_s[:, m:m + 1])
                # S8 (data-stationary): out_b = relu(fused @ Wf.T + bf),
                # producing the chunk in [b, H] rows for the gather.
                chunks = (
                    [(featsT[:, k, csl], 128) for k in range(KCH)]
                    + [(gloveT[:, csl], GEMB), (actE[:, csl], ADIM)]
                    + [(gcneT[:, k, csl], 128) for k in range(KCH)]
                )
                outps = ps_big(CW, H)
                for i, (lhsT, ksz) in enumerate(chunks):
                    nc.tensor.matmul(outps[:], lhsT, wfT[0:ksz, i, :],
                                     start=(i == 0), stop=(i == 9))
                outpre = smp.tile([CW, H], FP32, name="outpre", tag="outpre")
                nc.vector.tensor_tensor(outpre[:], outps[:], bfrep[:],
                                        ALU.add)
                outb = smp.tile([CW, H], BF16, name="outb", tag="outb")
                nc.vector.tensor_scalar_max(outb[:], outpre[:], 0.0)
                # AllGather the fused output chunk (b-major rows, bf16)
                oc_in = drp.tile([CW, H], BF16, name=f"oc_in{ck}",
                                 tag=f"oc_in{ck}")
                oc_all = drp.tile([CW * NCORES, H], BF16, name=f"oc_all{ck}",
                                  tag=f"oc_all{ck}", addr_space="Shared")
                nc.sync.dma_start(oc_in[:], outb[:])
                nc.gpsimd.collective_compute(
                    "AllGather", ALU.bypass,
                    replica_groups=[list(range(NCORES))],
                    ins=[oc_in.opt()], outs=[oc_all.opt()])
                # transpose the gathered chunk into [H, b] layout
                outTg = smp.tile([128, KCH, CW * NCORES], BF16, name="outTg",
                                 tag="outTg")
                for k in range(KCH):
                    eng = nc.sync if k % 2 == 0 else nc.scalar
                    eng.dma_start_transpose(outTg[:, k, :],
                                            oc_all[:, bass.ts(k, 128)])
                # S9 (weight-stationary, replicated): GxT chunk directly in
                # gate-major layout, bias folded into the PSUM->SBUF copy.
                gxt = pp.tile([128, GCH, BS, TCK], BF16, name=f"gxT{ck}",
                              tag=f"gxT{ck}")
                for gc in range(GCH):
                    gxtp = ps_big(128, CW * NCORES)
                    for k in range(KCH):
                        nc.tensor.matmul(gxtp[:], wihT[:, k, bass.ts(gc, 128)],
                                         outTg[:, k, :],
                                         start=(k == 0), stop=(k == KCH - 1))
                    nc.scalar.activation(
                        gxt[:, gc, :, :],
                        gxtp[:].rearrange("p (s t) -> p s t", t=TCK),
                        AF.Identity, bias=bih2c[:, gc:gc + 1])
                gxTc.append(gxt)

            # ---------- S10: LSTM over T steps (transposed form) ----------
            # gate blocks (permuted): i = gc 0:4, f = 4:8, o = 8:12, g = 12:16
            c_sb = pp.tile([128, KCH, BS], FP32, name="c_sb", tag="c_sb")
            hs = pp.tile([128, T, KCH, BS], BF16, name="hs", tag="hs")
            hfin = pp.tile([128, KCH, BS], FP32, name="hfin", tag="hfin")
            nc.vector.memset(c_sb[:], 0.0)
            for t in range(T):
                gxslice = gxTc[t // TCK][:, :, :, t % TCK]
                gp = ps.tile([128, GCH, BS], FP32, name="gp", tag="ps_big",
                             bufs=8)
                # Gx_t (+ bias) seeds the accumulator via an identity matmul
                # (start=True), then the Whh matmuls accumulate. The o-gate
                # block (gc 8:12) is computed last so its matmuls overlap the
                # i/f/g nonlinearities and the c update.
                nc.tensor.matmul(gp[:], eye[:], gxslice,
                                 start=True, stop=(t == 0),
                                 skip_group_check=True)
                gc_order = [0, 1, 2, 3, 4, 5, 6, 7, 12, 13, 14, 15,
                            8, 9, 10, 11]
                for gc in (gc_order if t > 0 else []):
                    for k in range(KCH):
                        nc.tensor.matmul(
                            gp[:, gc, :],
                            whhT[:, k, bass.ts(gc, 128)],
                            hs[:, t - 1, k, :],
                            start=False, stop=(k == KCH - 1),
                            skip_group_check=True)
                gates = smp.tile([128, GCH, BS], FP32, name="gates",
                                 tag="gates")
                nc.scalar.activation(gates[:, 0:8, :], gp[:, 0:8, :],
                                     AF.Sigmoid)
                nc.scalar.activation(gates[:, 12:16, :], gp[:, 12:16, :],
                                     AF.Tanh)
                t1 = smp.tile([128, KCH, BS], FP32, name="t1", tag="t1")
                t2 = smp.tile([128, KCH, BS], FP32, name="t2", tag="t2")
                nc.vector.tensor_tensor(t1[:], gates[:, 4:8, :], c_sb[:],
                                        ALU.mult)
                nc.vector.tensor_tensor(t2[:], gates[:, 0:4, :],
                                        gates[:, 12:16, :], ALU.mult)
                nc.vector.tensor_tensor(c_sb[:], t1[:], t2[:], ALU.add)
                tanhc = smp.tile([128, KCH, BS], FP32, name="tanhc",
                                 tag="tanhc")
                nc.scalar.activation(tanhc[:], c_sb[:], AF.Tanh)
                nc.scalar.activation(gates[:, 8:12, :], gp[:, 8:12, :],
                                     AF.Sigmoid)
                nc.vector.tensor_tensor(hs[:, t, :, :], gates[:, 8:12, :],
                                        tanhc[:], ALU.mult)
                if t == T - 1:
                    nc.vector.tensor_tensor(hfin[:], gates[:, 8:12, :],
                                            tanhc[:], ALU.mult)

            # ---------- S11: actor / critic heads (replicated, all B) -------
            def head(w1t, b1, w2t, b2, odim, out_dram, nm):
                h1ps = ps_big(128, B)
                for k in range(KCH):
                    nc.tensor.matmul(
                        h1ps[:], w1t[:, k, :],
                        hs[:, :, k, :].rearrange("p t s -> p s t"),
                        start=(k == 0), stop=(k == KCH - 1))
                h1 = smp.tile([128, B], FP32, name=f"h1{nm}", tag="h1")
                nc.scalar.activation(h1[:], h1ps[:], AF.Relu, bias=b1[:])
                h2ps = ps_big(odim, B)
                nc.tensor.matmul(h2ps[:], w2t[:, 0:odim], h1[:],
                                 start=True, stop=True)
                h2 = smp.tile([odim, B], FP32, name=f"h2{nm}", tag="h2")
                nc.scalar.activation(h2[:], h2ps[:], AF.Relu, bias=b2[:])
                nc.sync.dma_start(out_dram[:], h2[:])

            head(wa1T, ba1_s, wa2T, ba2_s, ASPACE, actorT_o, "a")
            head(wc1T, bc1_s, wc2T, bc2_s, 1, criticT_o, "c")

            nc.sync.dma_start(hT_o[:], hfin[:].rearrange("p a b -> p (a b)"))
            nc.sync.dma_start(cT_o[:], c_sb[:].rearrange("p a b -> p (a b)"))

    nc.compile()
    return nc


def _gate_perm():
    """Permutation of the 4H gate dim from torch order (i,f,g,o) to the
    kernel's (i,f,o,g)."""
    idx = np.arange(4 * H).reshape(4, H)
    return np.concatenate([idx[0], idx[1], idx[3], idx[2]])


def _host_prep(inp):
    f32 = np.float32
    bf16 = ml_dtypes.bfloat16
    target = np.asarray(inp["target"], f32).reshape(B, GLOVE)
    feats = np.asarray(inp["input_"], f32).reshape(B, RES)
    aprob = np.asarray(inp["action_probs"], f32).reshape(B, ASPACE)
    A = np.asarray(inp["A"], np.float64)
    all_glove = np.asarray(inp["all_glove"], np.float64)

    Wword = np.asarray(inp["Wword"], np.float64)
    bword = np.asarray(inp["bword"], np.float64)
    W0 = np.asarray(inp["W0"], np.float64)
    word = all_glove @ Wword.T + bword
    r = A.sum(1)
    v = (A @ word) @ W0[:, RES:].T
    vpT = (v / r[:, None]).T.astype(f32)
    apT = (A * r[None, :]).T.astype(f32)

    perm = _gate_perm()

    def padrows(x, n):
        out = np.zeros((n, x.shape[1]), f32)
        out[:x.shape[0]] = x
        return out

    Wf = np.asarray(inp["Wf"], f32)
    wfT = np.ascontiguousarray(Wf.T)
    wfT_pack = np.zeros((1280, H), f32)
    wfT_pack[0:512] = wfT[0:512]
    wfT_pack[512:512 + GEMB] = wfT[512:512 + GEMB]
    wfT_pack[640:640 + ADIM] = wfT[576:586]
    wfT_pack[768:1280] = wfT[586:1098]

    col = lambda x: np.ascontiguousarray(np.asarray(x, f32).reshape(-1, 1))
    tr = lambda x: np.ascontiguousarray(np.asarray(x, f32).T)

    def lay(x, a):
        """[(a*128), b] -> [128, a*b] chunk-major partition layout."""
        b_ = x.shape[1]
        return np.ascontiguousarray(
            x.reshape(a, 128, b_).transpose(1, 0, 2).reshape(128, a * b_))

    bihrow = (np.asarray(inp["bih"], f32)
              + np.asarray(inp["bhh"], f32))[perm].reshape(1, -1)
    common = {
        "w0fT": lay(W0[:, :RES].T.astype(f32), KCH),
        "wgT": lay(padrows(tr(inp["Wg"]), 384), 3),
        "waT": tr(inp["Wa"]),
        "vpT": lay(np.ascontiguousarray(vpT), 2).astype(bf16),
        "w1T": lay(tr(inp["W1"]), 2).astype(bf16),
        "apT": np.ascontiguousarray(apT).astype(bf16),
        "a_mat": A.astype(f32),
        "w2rep": np.ascontiguousarray(
            np.broadcast_to(np.asarray(inp["W2"], f32)[0], (NOBJ, C1))),
        "wfmT": tr(inp["Wfm"]),
        "wfT": lay(wfT_pack, 10),
        "wihT": lay(np.ascontiguousarray(tr(inp["Wih"])[:, perm]),
                    KCH).astype(bf16),
        "bfrep": np.ascontiguousarray(np.broadcast_to(
            np.asarray(inp["bf"], f32), (16, H))),
        "bih2c": np.ascontiguousarray(bihrow.reshape(GCH, 128).T),
        "whhT": lay(np.ascontiguousarray(tr(inp["Whh"])[:, perm]),
                    KCH).astype(bf16),
        "eye128": np.eye(128, dtype=f32).astype(bf16),
        "wa1T": lay(tr(inp["Wa1"]), KCH).astype(bf16),
        "wa2T": tr(inp["Wa2"]),
        "wc1T": lay(tr(inp["Wc1"]), KCH).astype(bf16),
        "wc2T": tr(inp["Wc2"]),
        "bg": col(inp["bg"]), "ba": col(inp["ba"]),
        "bfm": np.ascontiguousarray(
            np.asarray(inp["bfm"], f32).reshape(KCH, 128).T),
        "ba1": col(inp["ba1"]), "ba2": col(inp["ba2"]),
        "bc1": col(inp["bc1"]), "bc2": col(inp["bc2"]),
    }
    # local column order is chunk-major: col = ck*16 + s*8 + t8
    permb = np.array([s * 32 + ck * 8 + t
                      for ck in range(4) for s in range(2) for t in range(8)])
    in_maps = []
    for c in range(NCORES):
        sl = slice(c * BLOC, (c + 1) * BLOC)
        m = dict(common)
        m["featsT"] = lay(np.ascontiguousarray(feats[sl].T[:, permb]), KCH)
        m["tgtT"] = lay(padrows(
            np.ascontiguousarray(target[sl].T[:, permb]), 384), 3)
        m["actT"] = np.ascontiguousarray(aprob[sl].T[:, permb])
        in_maps.append(m)
    return in_maps


def _run(inp, trace=False, **kw):
    if "nc" not in _CACHE:
        _CACHE["nc"] = _build_program()
    nc = _CACHE["nc"]
    in_maps = _host_prep(inp)
    res = run_bass_kernel_spmd(nc, in_maps, list(range(NCORES)), trace=trace,
                               **kw)
    r0 = res.results[0]
    actor = np.ascontiguousarray(r0["actorT"].T).reshape(BS, T, ASPACE)
    critic = np.ascontiguousarray(r0["criticT"].T).reshape(BS, T, 1)
    hT = r0["hT"].reshape(128, KCH, BS).transpose(2, 1, 0).reshape(1, BS, H)
    cT = r0["cT"].reshape(128, KCH, BS).transpose(2, 1, 0).reshape(1, BS, H)
    return (actor.astype(np.float32), critic.astype(np.float32),
            np.ascontiguousarray(hT).astype(np.float32),
            np.ascontiguousarray(cT).astype(np.float32)), res


def kernel(**inputs):
    outs, _ = _run(inputs, trace=False)
    return outs


# revision 38
# speedup vs baseline: 1.3901x; 1.3287x over previous
"""Trainium2 Bass kernel for nn_GCN (batched GCN + LSTM + actor/critic heads).

Strategy (8 NeuronCores, SPMD):
  - Stage 1 (data-parallel over B = bs*T = 512; 64 rows per core):
    algebraically-collapsed GCN layer 1 (rank-structured broadcast + relu),
    per-b node matmuls for layers 2/3, feature heads, fused FC layer, and the
    LSTM input-gate precompute Gx = out @ Wih.T (+ bias via a ones-row).
  - Gx shards are AllGathered (bf16) across the 8 cores, then transposed into
    gate-major layout with xbar DMA transposes.
  - Stage 2 (replicated on every core): the sequential LSTM over T=32 steps in
    transposed (gate-on-partition) form with bf16 Whh stationary weights; Gx is
    injected into PSUM with an identity-matmul accumulate. Gates are laid out
    (i, f, o, g) so one sigmoid covers i/f/o. Hidden states are stored bf16 and
    feed the next step and the actor/critic heads directly.
Host side only reshapes/transposes/casts weights and derives B-independent
small matrices (word embedding, normalized adjacency variants).
"""

import numpy as np
import ml_dtypes

import concourse.bacc as bacc
import concourse.bass as bass
import concourse.tile as tile
import concourse.mybir as mybir
from concourse.bass_utils import run_bass_kernel_spmd

FP32 = mybir.dt.float32
BF16 = mybir.dt.bfloat16
AF = mybir.ActivationFunctionType
ALU = mybir.AluOpType

NCORES = 8
BS, T, NOBJ = 16, 32, 100
GLOVE, GEMB, ADIM, GCN_DIM, H, ASPACE, RES = 300, 64, 10, 512, 512, 6, 512
B = BS * T          # 512
BLOC = B // NCORES  # 64 rows per core
BBLK = 8            # b-block size for the Z/GCN pipeline
C1 = 256            # GCN hidden channels
KCH = H // 128      # 4
GCH = 4 * H // 128  # 16

_CACHE = {}
GX_VIA_MM = True


def _build_program():
    nc = bacc.Bacc("TRN2", target_bir_lowering=False, debug=False,
                   num_devices=NCORES)

    def din(name, shape, dt=FP32):
        return nc.dram_tensor(name, shape, dt, kind="ExternalInput").ap()

    def dout(name, shape, dt=FP32):
        return nc.dram_tensor(name, shape, dt, kind="ExternalOutput").ap()

    # --- per-core (sharded) batch inputs (pre-laid [128, chunks*cols]) ---
    featsT_d = din("featsT", [128, KCH * BLOC])
    tgtT_d = din("tgtT", [128, 3 * BLOC])
    actT_d = din("actT", [ASPACE, BLOC])
    # --- replicated weights / derived (gate dims permuted to i,f,o,g) ---
    w0fT_d = din("w0fT", [128, KCH * C1])
    wgT_d = din("wgT", [128, 3 * GEMB])
    waT_d = din("waT", [ASPACE, ADIM])
    vpT_d = din("vpT", [128, 2 * NOBJ], BF16)
    w1T_d = din("w1T", [128, 2 * C1], BF16)
    apT_d = din("apT", [NOBJ, NOBJ], BF16)
    a_d = din("a_mat", [NOBJ, NOBJ])
    w2r_d = din("w2rep", [NOBJ, C1])
    wfmT_d = din("wfmT", [NOBJ, GCN_DIM])
    wfT_d = din("wfT", [128, 10 * H])
    wihT_d = din("wihT", [128, KCH * 4 * H], BF16)
    bfrep_d = din("bfrep", [2 * T // 4, H])        # bf replicated rows
    bih2c_d = din("bih2c", [128, GCH])             # bih+bhh (permuted) chunks
    whhT_d = din("whhT", [128, KCH * 4 * H], BF16)
    eye_d = din("eye128", [128, 128], BF16)
    wa1T_d = din("wa1T", [128, KCH * 128], BF16)
    wa2T_d = din("wa2T", [128, ASPACE])
    wc1T_d = din("wc1T", [128, KCH * 128], BF16)
    wc2T_d = din("wc2T", [128, 1])
    bg_d = din("bg", [GEMB, 1])
    ba_d = din("ba", [ADIM, 1])
    bfm_d = din("bfm", [128, KCH])
    ba1_d = din("ba1", [128, 1])
    ba2_d = din("ba2", [ASPACE, 1])
    bc1_d = din("bc1", [128, 1])
    bc2_d = din("bc2", [1, 1])

    actorT_o = dout("actorT", [ASPACE, B])
    criticT_o = dout("criticT", [1, B])
    hT_o = dout("hT", [128, KCH * BS])
    cT_o = dout("cT", [128, KCH * BS])

    with tile.TileContext(nc) as tc:
        with (
            tc.tile_pool(name="persist", bufs=1) as pp,
            tc.tile_pool(name="wstream", bufs=2) as wsp,
            tc.tile_pool(name="zblk", bufs=3) as zbp,
            tc.tile_pool(name="small", bufs=2) as smp,
            tc.tile_pool(name="psum", bufs=2, space="PSUM") as ps,
            tc.tile_pool(name="dram", bufs=1, space="DRAM") as drp,
        ):
            def load(dram_ap, name, shape, dt=FP32):
                t_ = pp.tile(shape, dt, name=name, tag=name)
                dst = t_[:]
                if len(shape) == 3:
                    dst = dst.rearrange("p a b -> p (a b)")
                nc.sync.dma_start(dst, dram_ap)
                return t_

            def ps_sm(p_, f_):
                return ps.tile([p_, f_], FP32, name="ps_sm", tag="ps_big",
                               bufs=8)

            def ps_big(p_, f_, f2=None):
                shp = [p_, f_] if f2 is None else [p_, f_, f2]
                return ps.tile(shp, FP32, name="ps_big", tag="ps_big", bufs=8)

            # stage-1-critical loads first
            featsT = load(featsT_d, "featsT_s", [128, KCH, BLOC])
            w0fT = load(w0fT_d, "w0fT_s", [128, KCH, C1])
            vpT = load(vpT_d, "vpT_s", [128, 2, NOBJ], dt=BF16)
            w1T = load(w1T_d, "w1T_s", [128, 2, C1], dt=BF16)
            apT = load(apT_d, "apT_s", [NOBJ, NOBJ], dt=BF16)
            w2rep = load(w2r_d, "w2rep_s", [NOBJ, C1])
            tgtT = load(tgtT_d, "tgtT_s", [128, 3, BLOC])
            wgT = load(wgT_d, "wgT_s", [128, 3, GEMB])
            actT = load(actT_d, "actT_s", [ASPACE, BLOC])
            waT = load(waT_d, "waT_s", [ASPACE, ADIM])
            a_sb = load(a_d, "a_s", [NOBJ, NOBJ])
            wfmT = load(wfmT_d, "wfmT_s", [NOBJ, GCN_DIM])
            bg_s = load(bg_d, "bg_s", [GEMB, 1])
            ba_s = load(ba_d, "ba_s", [ADIM, 1])
            bfm_s = load(bfm_d, "bfm_s", [128, KCH])

            # ---------- S1: u = feats @ W0f.T -> uT (fp32) -> ubf (bf16) ----
            uT = pp.tile([128, 2, BLOC], FP32, name="uT", tag="uT")
            for m in range(2):
                ups = ps_sm(128, BLOC)
                for k in range(KCH):
                    nc.tensor.matmul(ups[:], w0fT[:, k, bass.ts(m, 128)],
                                     featsT[:, k, :],
                                     start=(k == 0), stop=(k == KCH - 1))
                nc.scalar.copy(uT[:, m, :], ups[:])
            ubf = pp.tile([128, 2, BLOC], BF16, name="ubf", tag="ubf")
            nc.vector.tensor_copy(ubf[:], uT[:])

            # ---------- S2/S3: glove_e, act_e ----------
            gloveT = pp.tile([GEMB, BLOC], FP32, name="gloveT", tag="gloveT")
            gps = ps_sm(GEMB, BLOC)
            for k in range(3):
                nc.tensor.matmul(gps[:], wgT[:, k, :], tgtT[:, k, :],
                                 start=(k == 0), stop=(k == 2))
            nc.scalar.activation(gloveT[:], gps[:], AF.Relu, bias=bg_s[:])

            actE = pp.tile([ADIM, BLOC], FP32, name="actE", tag="actE")
            aps2 = ps_sm(ADIM, BLOC)
            nc.tensor.matmul(aps2[:], waT[:], actT[:], start=True, stop=True)
            nc.scalar.activation(actE[:], aps2[:], AF.Relu, bias=ba_s[:])

            # ---------- late loads (LSTM / heads weights) ----------
            wfT = load(wfT_d, "wfT_s", [128, 10, H])
            wihT = load(wihT_d, "wihT_s", [128, KCH, 4 * H], dt=BF16)
            whhT = load(whhT_d, "whhT_s", [128, KCH, 4 * H], dt=BF16)
            eye = load(eye_d, "eye_s", [128, 128], dt=BF16)
            bfrep = load(bfrep_d, "bfrep_s", [2 * T // 4, H])
            bih2c = load(bih2c_d, "bih2c_s", [128, GCH])
            wa1T = load(wa1T_d, "wa1T_s", [128, KCH, 128], dt=BF16)
            wa2T = load(wa2T_d, "wa2T_s", [128, ASPACE])
            wc1T = load(wc1T_d, "wc1T_s", [128, KCH, 128], dt=BF16)
            wc2T = load(wc2T_d, "wc2T_s", [128, 1])
            ba1_s = load(ba1_d, "ba1_s", [128, 1])
            ba2_s = load(ba2_d, "ba2_s", [ASPACE, 1])
            bc1_s = load(bc1_d, "bc1_s", [128, 1])
            bc2_s = load(bc2_d, "bc2_s", [1, 1])

            # ---------- S4..S9 pipelined over 4 t-chunks of 8 steps --------
            # Local b for chunk ck: {s*32 + 8*ck + i : s in 0..1, i in 0..7}.
            # Each chunk runs GCN layers 2/3, heads-fusion, Gx, and its own
            # AllGather + gate-transposes so the LSTM can start after chunk 0.
            # Local b columns are chunk-major: col = ck*16 + s*8 + t8
            # (host permutes featsT/tgtT/actT accordingly), so every chunk's
            # 16 columns are contiguous.
            NCK, TCK = 4, T // 4
            CW = 2 * TCK                       # 16 columns per chunk
            mcol = pp.tile([NOBJ, BLOC], FP32, name="mcol", tag="mcol")
            x3T = pp.tile([NOBJ, BLOC], FP32, name="x3T", tag="x3T")
            gcneT = pp.tile([128, KCH, BLOC], FP32, name="gcneT", tag="gcneT")
            zts = {}
            oc_alls = {}
            gxTc = {}

            def z_piece(ck, cc):
                csl = bass.ts(ck, CW)
                if cc == 0:
                    zts[ck] = zbp.tile([128, 2, CW, NOBJ], BF16, name="zt",
                                       tag="zt")
                zpre = smp.tile([128, CW, NOBJ], BF16, name="zpre", tag="zpre")
                nc.gpsimd.tensor_tensor(
                    zpre[:],
                    ubf[:, cc, csl, None].to_broadcast([128, CW, NOBJ]),
                    vpT[:, cc, None, :].to_broadcast([128, CW, NOBJ]),
                    ALU.add)
                nc.vector.tensor_scalar_max(zts[ck][:, cc, :, :], zpre[:], 0.0)

            def b_piece(ck, bb):
                zt = zts[ck]
                b = ck * CW + bb
                pps = ps_big(NOBJ, C1)
                nc.tensor.matmul(pps[:], zt[:, 0, bb, :],
                                 w1T[:, 0, :], start=True, stop=False)
                nc.tensor.matmul(pps[:], zt[:, 1, bb, :],
                                 w1T[:, 1, :], start=False, stop=True)
                psb = smp.tile([NOBJ, C1], BF16, name="psb", tag="psb")
                nc.scalar.copy(psb[:], pps[:])
                x2ps = ps_big(NOBJ, C1)
                nc.tensor.matmul(x2ps[:], apT[:], psb[:],
                                 start=True, stop=True)
                scr = smp.tile([NOBJ, C1], FP32, name="scr", tag="scr")
                nc.vector.scalar_tensor_tensor(
                    scr[:], x2ps[:], 0.0, w2rep[:],
                    op0=ALU.max, op1=ALU.mult,
                    accum_out=mcol[:, b:b + 1])

            def tail_piece(ck):
                csl = bass.ts(ck, CW)
                # S6: x3 chunk = relu(A @ mcol[:, chunk cols])
                x3ps = ps_sm(NOBJ, CW)
                nc.tensor.matmul(x3ps[:], a_sb[:], mcol[:, csl],
                                 start=True, stop=True)
                nc.scalar.activation(x3T[:, csl], x3ps[:], AF.Relu)
                # S7: gcn_e chunk
                for m in range(KCH):
                    gps2 = ps_sm(128, CW)
                    nc.tensor.matmul(gps2[:], wfmT[:, bass.ts(m, 128)],
                                     x3T[:, csl], start=True, stop=True)
                    nc.scalar.activation(gcneT[:, m, csl], gps2[:],
                                         AF.Identity, bias=bfm_s[:, m:m + 1])
                # S8 (data-stationary): out_b rows for the gather
                chunks = (
                    [(featsT[:, k, csl], 128) for k in range(KCH)]
                    + [(gloveT[:, csl], GEMB), (actE[:, csl], ADIM)]
                    + [(gcneT[:, k, csl], 128) for k in range(KCH)]
                )
                outps = ps_big(CW, H)
                for i, (lhsT, ksz) in enumerate(chunks):
                    nc.tensor.matmul(outps[:], lhsT, wfT[0:ksz, i, :],
                                     start=(i == 0), stop=(i == 9))
                outpre = smp.tile([CW, H], FP32, name="outpre", tag="outpre")
                nc.vector.tensor_tensor(outpre[:], outps[:], bfrep[:],
                                        ALU.add)
                outb = smp.tile([CW, H], BF16, name="outb", tag="outb")
                nc.vector.tensor_scalar_max(outb[:], outpre[:], 0.0)
                # AllGather the fused output chunk (b-major rows, bf16)
                oc_in = drp.tile([CW, H], BF16, name=f"oc_in{ck}",
                                 tag=f"oc_in{ck}")
                oc_all = drp.tile([CW * NCORES, H], BF16, name=f"oc_all{ck}",
                                  tag=f"oc_all{ck}", addr_space="Shared")
                nc.sync.dma_start(oc_in[:], outb[:])
                nc.gpsimd.collective_compute(
                    "AllGather", ALU.bypass,
                    replica_groups=[list(range(NCORES))],
                    ins=[oc_in.opt()], outs=[oc_all.opt()])
                oc_alls[ck] = oc_all

            def gxt_piece(ck):
                # transpose gathered chunk into [H, b], then GxT chunk via
                # weight-stationary Wih (gate-major out, bias in the copy).
                oc_all = oc_alls[ck]
                outTg = smp.tile([128, KCH, CW * NCORES], BF16, name="outTg",
                                 tag="outTg")
                for k in range(KCH):
                    eng = nc.sync if k % 2 == 0 else nc.scalar
                    eng.dma_start_transpose(outTg[:, k, :],
                                            oc_all[:, bass.ts(k, 128)])
                gxt = pp.tile([128, GCH, BS, TCK], BF16, name=f"gxT{ck}",
                              tag=f"gxT{ck}")
                for gc in range(GCH):
                    gxtp = ps_big(128, CW * NCORES)
                    for k in range(KCH):
                        nc.tensor.matmul(gxtp[:],
                                         wihT[:, k, bass.ts(gc, 128)],
                                         outTg[:, k, :],
                                         start=(k == 0), stop=(k == KCH - 1))
                    nc.scalar.activation(
                        gxt[:, gc, :, :],
                        gxtp[:].rearrange("p (s t) -> p s t", t=TCK),
                        AF.Identity, bias=bih2c[:, gc:gc + 1])
                gxTc[ck] = gxt

            def chunk_pieces(ck):
                return ([lambda cc=cc: z_piece(ck, cc) for cc in range(2)]
                        + [lambda bb=bb: b_piece(ck, bb) for bb in range(CW)]
                        + [lambda: tail_piece(ck)])

            # ---------- S10: LSTM, with chunks 2-3 interleaved ----------
            # gate blocks (permuted): i = gc 0:4, f = 4:8, o = 8:12, g = 12:16
            c_sb = pp.tile([128, KCH, BS], FP32, name="c_sb", tag="c_sb")
            hs = pp.tile([128, T, KCH, BS], BF16, name="hs", tag="hs")
            hfin = pp.tile([128, KCH, BS], FP32, name="hfin", tag="hfin")
            nc.vector.memset(c_sb[:], 0.0)

            for piece in chunk_pieces(0) + chunk_pieces(1):
                piece()
            gxt_piece(0)
            pending = chunk_pieces(2) + chunk_pieces(3)
            DRAIN_STEPS = 12
            PTOT = len(pending)
            emitted = 0

            for t in range(T):
                if t > 0 and t % TCK == 0:
                    gxt_piece(t // TCK)
                gxslice = gxTc[t // TCK][:, :, :, t % TCK]
                gp = ps.tile([128, GCH, BS], FP32, name="gp", tag="ps_big",
                             bufs=8)
                # Gx_t (+ bias) seeds the accumulator via an identity matmul
                # (start=True), then the Whh matmuls accumulate. The o-gate
                # block (gc 8:12) is computed last so its matmuls overlap the
                # i/f/g nonlinearities and the c update.
                nc.tensor.matmul(gp[:], eye[:], gxslice,
                                 start=True, stop=(t == 0),
                                 skip_group_check=True)
                gc_order = [0, 1, 2, 3, 4, 5, 6, 7, 12, 13, 14, 15,
                            8, 9, 10, 11]
                for gc in (gc_order if t > 0 else []):
                    for k in range(KCH):
                        nc.tensor.matmul(
                            gp[:, gc, :],
                            whhT[:, k, bass.ts(gc, 128)],
                            hs[:, t - 1, k, :],
                            start=False, stop=(k == KCH - 1),
                            skip_group_check=True)
                gates = smp.tile([128, GCH, BS], FP32, name="gates",
                                 tag="gates")
                nc.scalar.activation(gates[:, 0:8, :], gp[:, 0:8, :],
                                     AF.Sigmoid)
                nc.scalar.activation(gates[:, 12:16, :], gp[:, 12:16, :],
                                     AF.Tanh)
                t1 = smp.tile([128, KCH, BS], FP32, name="t1", tag="t1")
                t2 = smp.tile([128, KCH, BS], FP32, name="t2", tag="t2")
                nc.vector.tensor_tensor(t1[:], gates[:, 4:8, :], c_sb[:],
                                        ALU.mult)
                nc.vector.tensor_tensor(t2[:], gates[:, 0:4, :],
                                        gates[:, 12:16, :], ALU.mult)
                nc.vector.tensor_tensor(c_sb[:], t1[:], t2[:], ALU.add)
                tanhc = smp.tile([128, KCH, BS], FP32, name="tanhc",
                                 tag="tanhc")
                nc.scalar.activation(tanhc[:], c_sb[:], AF.Tanh)
                nc.scalar.activation(gates[:, 8:12, :], gp[:, 8:12, :],
                                     AF.Sigmoid)
                nc.vector.tensor_tensor(hs[:, t, :, :], gates[:, 8:12, :],
                                        tanhc[:], ALU.mult)
                if t == T - 1:
                    nc.vector.tensor_tensor(hfin[:], gates[:, 8:12, :],
                                            tanhc[:], ALU.mult)
                # drain stage-1 pieces for chunks 2-3 into the step gaps
                quota = min(PTOT, ((t + 1) * PTOT) // DRAIN_STEPS)
                while emitted < quota:
                    pending[emitted]()
                    emitted += 1

            # ---------- S11: actor / critic heads (replicated, all B) -------
            def head(w1t, b1, w2t, b2, odim, out_dram, nm):
                h1ps = ps_big(128, B)
                for k in range(KCH):
                    nc.tensor.matmul(
                        h1ps[:], w1t[:, k, :],
                        hs[:, :, k, :].rearrange("p t s -> p s t"),
                        start=(k == 0), stop=(k == KCH - 1))
                h1 = smp.tile([128, B], FP32, name=f"h1{nm}", tag="h1")
                nc.scalar.activation(h1[:], h1ps[:], AF.Relu, bias=b1[:])
                h2ps = ps_big(odim, B)
                nc.tensor.matmul(h2ps[:], w2t[:, 0:odim], h1[:],
                                 start=True, stop=True)
                h2 = smp.tile([odim, B], FP32, name=f"h2{nm}", tag="h2")
                nc.scalar.activation(h2[:], h2ps[:], AF.Relu, bias=b2[:])
                nc.sync.dma_start(out_dram[:], h2[:])

            head(wa1T, ba1_s, wa2T, ba2_s, ASPACE, actorT_o, "a")
            head(wc1T, bc1_s, wc2T, bc2_s, 1, criticT_o, "c")

            nc.sync.dma_start(hT_o[:], hfin[:].rearrange("p a b -> p (a b)"))
            nc.sync.dma_start(cT_o[:], c_sb[:].rearrange("p a b -> p (a b)"))

    nc.compile()
    return nc


def _gate_perm():
    """Permutation of the 4H gate dim from torch order (i,f,g,o) to the
    kernel's (i,f,o,g)."""
    idx = np.arange(4 * H).reshape(4, H)
    return np.concatenate([idx[0], idx[1], idx[3], idx[2]])


def _host_prep(inp):
    f32 = np.float32
    bf16 = ml_dtypes.bfloat16
    target = np.asarray(inp["target"], f32).reshape(B, GLOVE)
    feats = np.asarray(inp["input_"], f32).reshape(B, RES)
    aprob = np.asarray(inp["action_probs"], f32).reshape(B, ASPACE)
    A = np.asarray(inp["A"], np.float64)
    all_glove = np.asarray(inp["all_glove"], np.float64)

    Wword = np.asarray(inp["Wword"], np.float64)
    bword = np.asarray(inp["bword"], np.float64)
    W0 = np.asarray(inp["W0"], np.float64)
    word = all_glove @ Wword.T + bword
    r = A.sum(1)
    v = (A @ word) @ W0[:, RES:].T
    vpT = (v / r[:, None]).T.astype(f32)
    apT = (A * r[None, :]).T.astype(f32)

    perm = _gate_perm()

    def padrows(x, n):
        out = np.zeros((n, x.shape[1]), f32)
        out[:x.shape[0]] = x
        return out

    Wf = np.asarray(inp["Wf"], f32)
    wfT = np.ascontiguousarray(Wf.T)
    wfT_pack = np.zeros((1280, H), f32)
    wfT_pack[0:512] = wfT[0:512]
    wfT_pack[512:512 + GEMB] = wfT[512:512 + GEMB]
    wfT_pack[640:640 + ADIM] = wfT[576:586]
    wfT_pack[768:1280] = wfT[586:1098]

    col = lambda x: np.ascontiguousarray(np.asarray(x, f32).reshape(-1, 1))
    tr = lambda x: np.ascontiguousarray(np.asarray(x, f32).T)

    def lay(x, a):
        """[(a*128), b] -> [128, a*b] chunk-major partition layout."""
        b_ = x.shape[1]
        return np.ascontiguousarray(
            x.reshape(a, 128, b_).transpose(1, 0, 2).reshape(128, a * b_))

    bihrow = (np.asarray(inp["bih"], f32)
              + np.asarray(inp["bhh"], f32))[perm].reshape(1, -1)
    common = {
        "w0fT": lay(W0[:, :RES].T.astype(f32), KCH),
        "wgT": lay(padrows(tr(inp["Wg"]), 384), 3),
        "waT": tr(inp["Wa"]),
        "vpT": lay(np.ascontiguousarray(vpT), 2).astype(bf16),
        "w1T": lay(tr(inp["W1"]), 2).astype(bf16),
        "apT": np.ascontiguousarray(apT).astype(bf16),
        "a_mat": A.astype(f32),
        "w2rep": np.ascontiguousarray(
            np.broadcast_to(np.asarray(inp["W2"], f32)[0], (NOBJ, C1))),
        "wfmT": tr(inp["Wfm"]),
        "wfT": lay(wfT_pack, 10),
        "wihT": lay(np.ascontiguousarray(tr(inp["Wih"])[:, perm]),
                    KCH).astype(bf16),
        "bfrep": np.ascontiguousarray(np.broadcast_to(
            np.asarray(inp["bf"], f32), (16, H))),
        "bih2c": np.ascontiguousarray(bihrow.reshape(GCH, 128).T),
        "whhT": lay(np.ascontiguousarray(tr(inp["Whh"])[:, perm]),
                    KCH).astype(bf16),
        "eye128": np.eye(128, dtype=f32).astype(bf16),
        "wa1T": lay(tr(inp["Wa1"]), KCH).astype(bf16),
        "wa2T": tr(inp["Wa2"]),
        "wc1T": lay(tr(inp["Wc1"]), KCH).astype(bf16),
        "wc2T": tr(inp["Wc2"]),
        "bg": col(inp["bg"]), "ba": col(inp["ba"]),
        "bfm": np.ascontiguousarray(
            np.asarray(inp["bfm"], f32).reshape(KCH, 128).T),
        "ba1": col(inp["ba1"]), "ba2": col(inp["ba2"]),
        "bc1": col(inp["bc1"]), "bc2": col(inp["bc2"]),
    }
    # local column order is chunk-major: col = ck*16 + s*8 + t8
    permb = np.array([s * 32 + ck * 8 + t
                      for ck in range(4) for s in range(2) for t in range(8)])
    in_maps = []
    for c in range(NCORES):
        sl = slice(c * BLOC, (c + 1) * BLOC)
        m = dict(common)
        m["featsT"] = lay(np.ascontiguousarray(feats[sl].T[:, permb]), KCH)
        m["tgtT"] = lay(padrows(
            np.ascontiguousarray(target[sl].T[:, permb]), 384), 3)
        m["actT"] = np.ascontiguousarray(aprob[sl].T[:, permb])
        in_maps.append(m)
    return in_maps


def _run(inp, trace=False, **kw):
    if "nc" not in _CACHE:
        _CACHE["nc"] = _build_program()
    nc = _CACHE["nc"]
    in_maps = _host_prep(inp)
    res = run_bass_kernel_spmd(nc, in_maps, list(range(NCORES)), trace=trace,
                               **kw)
    r0 = res.results[0]
    actor = np.ascontiguousarray(r0["actorT"].T).reshape(BS, T, ASPACE)
    critic = np.ascontiguousarray(r0["criticT"].T).reshape(BS, T, 1)
    hT = r0["hT"].reshape(128, KCH, BS).transpose(2, 1, 0).reshape(1, BS, H)
    cT = r0["cT"].reshape(128, KCH, BS).transpose(2, 1, 0).reshape(1, BS, H)
    return (actor.astype(np.float32), critic.astype(np.float32),
            np.ascontiguousarray(hT).astype(np.float32),
            np.ascontiguousarray(cT).astype(np.float32)), res


def kernel(**inputs):
    outs, _ = _run(inputs, trace=False)
    return outs
